# revision 29
# baseline (speedup 1.0000x reference)
"""Differentiable JPEG on 8 Trainium2 NeuronCores (Bass/Tile).

Full inputs in, full outputs out. Data-parallel over the batch dim:
32 images -> 8 cores x 4 images (12 channel-planes of 512x512 each).

Per 512x512 plane P (values in [0,1]):
  Y^T = A^T (X^T A)          A = kron(I16, D^T)   (2D DCT, transposed layout;
                                                   both stages fp32 matmuls,
                                                   S1 uses the data as lhsT so
                                                   the transpose comes free)
  n   = Y^T * (1/q)          (DVE tensor_tensor, reads PSUM)
  n_dc -= 4/q00              (strided DC fix: folds the reference's x-0.5)
  r   = (n + C) - C          C = 1.5*2^23: fp32 magic-constant round-half-even
  d   = r * q                (GpSimd scalar_tensor_tensor, bf16 out -- exact:
                              |r|<=1 and q in {k/2} so r*q fits bf16)
  W   = d^T B                B = kron(I16, D), split B = Bhi+Blo in bf16
  W  += sqrt2 on rows p%8==0 (ACT Identity bias: folds the +4 DC / +0.5 shift)
  S   = B^T W                (fp32 matmul)
  out = min(relu(S), 1)
"""

import numpy as np

import concourse.bass as bass
import concourse.bacc as bacc
import concourse.mybir as mybir
import concourse.tile as tile

N_CORES = 8
B_FULL, C_CH, H, W = 32, 3, 512, 512
PER_CORE_B = B_FULL // N_CORES          # 4
N_IMG = PER_CORE_B * C_CH               # 12 planes of 512x512 per core
MAGIC = 12582912.0                      # 1.5 * 2**23
F32 = mybir.dt.float32
BF16 = mybir.dt.bfloat16

_JPEG_QUANT_TABLE = np.array([
    [16, 11, 10, 16, 24, 40, 51, 61],
    [12, 12, 14, 19, 26, 58, 60, 55],
    [14, 13, 16, 24, 40, 57, 69, 56],
    [14, 17, 22, 29, 51, 87, 80, 62],
    [18, 22, 37, 56, 68, 109, 103, 77],
    [24, 35, 55, 64, 81, 104, 113, 92],
    [49, 64, 78, 87, 103, 121, 120, 101],
    [72, 92, 95, 98, 112, 100, 103, 99]], dtype=np.float32)


def _dct_matrix():
    n = 8
    k = np.arange(n)[:, None].astype(np.float64)
    i = np.arange(n)[None, :].astype(np.float64)
    alpha = np.where(k == 0, np.sqrt(1.0 / n), np.sqrt(2.0 / n))
    return (alpha * np.cos(np.pi * (2 * i + 1) * k / (2 * n))).astype(np.float32)


def host_constants(quality: int):
    D = _dct_matrix()
    scale = 5000.0 / quality if quality < 50 else 200.0 - 2.0 * quality
    qt = np.clip((_JPEG_QUANT_TABLE * np.float32(scale)) / np.float32(100.0),
                 np.float32(1.0), np.float32(255.0)).astype(np.float32)
    qinv = (np.float32(1.0) / qt).astype(np.float32)

    import ml_dtypes
    I16 = np.eye(16, dtype=np.float32)
    A128 = np.kron(I16, D.T).astype(np.float32)          # S1 rhs / S2 lhsT
    B128 = np.kron(I16, D).astype(np.float32)            # S3 rhs / S4 lhsT
    Abf = A128.astype(ml_dtypes.bfloat16)
    Bhi = B128.astype(ml_dtypes.bfloat16)
    Blo = (B128 - Bhi.astype(np.float32)).astype(ml_dtypes.bfloat16)

    # [128, 4*512] patterns over the Y^T layout: [p, j] -> q[j%8, p%8]
    p_idx = np.arange(128) % 8
    j_idx = np.arange(4 * 512) % 8
    QINV = np.ascontiguousarray(qinv[np.ix_(j_idx, p_idx)].T)   # [128, 2048]
    QQ = np.ascontiguousarray(qt[np.ix_(j_idx, p_idx)].T)       # [128, 2048]

    # rank-1 DC shift injected into ps2: Y^T - 4*P4 (folds the x-0.5 input
    # shift). u (lhsT, [1,128]) and v (rhs, [1,512]) are bf16-exact.
    DCU = np.where(np.arange(128) % 8 == 0, -4.0, 0.0)[None, :] \
        .astype(ml_dtypes.bfloat16)
    DCV = np.where(np.arange(512) % 8 == 0, 1.0, 0.0)[None, :] \
        .astype(ml_dtypes.bfloat16)
    dcb_val = np.float32(0.5 / np.float64(D[0, 0]))              # = sqrt(2)
    DCB = np.where(np.arange(128) % 8 == 0, dcb_val,
                   np.float32(0)).astype(np.float32)[:, None]    # [128, 1]
    return dict(A128=A128, B128=B128, Abf=np.asarray(Abf),
                Bhi=np.asarray(Bhi), Blo=np.asarray(Blo),
                QINV=QINV, QQ=QQ, DCB=DCB,
                DCU=np.asarray(DCU), DCV=np.asarray(DCV))


def build_program(quality: int):
    """Build + compile the per-core Bass program. Returns (nc, const arrays)."""
    consts = host_constants(quality)
    nc = bacc.Bacc("TRN2", target_bir_lowering=False, debug=False,
                   num_devices=N_CORES)

    x_d = nc.dram_tensor("x", [N_IMG, H, W], BF16, kind="ExternalInput")
    y_d = nc.dram_tensor("y", [N_IMG, H, W], F32, kind="ExternalOutput")
    abf_d = nc.dram_tensor("abf", [128, 128], BF16, kind="ExternalInput")
    b_d = nc.dram_tensor("b128", [128, 128], F32, kind="ExternalInput")
    bhi_d = nc.dram_tensor("bhi", [128, 128], BF16, kind="ExternalInput")
    blo_d = nc.dram_tensor("blo", [128, 128], BF16, kind="ExternalInput")
    qi_d = nc.dram_tensor("qinv", [128, 4, 512], F32, kind="ExternalInput")
    qq_d = nc.dram_tensor("qq", [128, 4, 512], F32, kind="ExternalInput")
    db_d = nc.dram_tensor("dcb", [128, 1], F32, kind="ExternalInput")
    dcu_d = nc.dram_tensor("dcu", [1, 128], BF16, kind="ExternalInput")
    dcv_d = nc.dram_tensor("dcv", [1, 512], BF16, kind="ExternalInput")

    op = mybir.AluOpType

    with tile.TileContext(nc) as tc:
        with (
            tc.tile_pool(name="const", bufs=1) as cpool,
            tc.tile_pool(name="sb", bufs=3) as pool,
            tc.tile_pool(name="ps", bufs=8, space="PSUM") as pspool,
        ):
            Abf = cpool.tile([128, 128], BF16)
            Bf = cpool.tile([128, 128], F32)
            Bhi = cpool.tile([128, 128], BF16)
            Blo = cpool.tile([128, 128], BF16)
            QI = cpool.tile([128, 4, 512], F32)
            QQ = cpool.tile([128, 4, 512], F32)
            DCB = cpool.tile([128, 1], F32)
            DCU = cpool.tile([1, 128], BF16)
            DCV = cpool.tile([1, 512], BF16)
            nc.sync.dma_start(DCU[:], dcu_d.ap())
            nc.sync.dma_start(DCV[:], dcv_d.ap())
            nc.sync.dma_start(Abf[:], abf_d.ap())
            nc.sync.dma_start(Bf[:], b_d.ap())
            nc.sync.dma_start(Bhi[:], bhi_d.ap())
            nc.sync.dma_start(Blo[:], blo_d.ap())
            nc.sync.dma_start(QI[:], qi_d.ap())
            nc.sync.dma_start(QQ[:], qq_d.ap())
            nc.sync.dma_start(DCB[:], db_d.ap())

            # ---- software pipeline: tick i runs {S1,S2,quant} of image
            # a=i and {S3,S4,clip,out} of image b=i-2. Phase A uses
            # ps3(b)+ps1(a) = 8 banks, phase B ps4(b)+ps2(a) = 8 banks, so
            # the PE never waits on a PSUM slot and stays dense (HAM-warm).
            T = {}

            def dma_in(a):
                xv = x_d.ap()[a].rearrange("(cb p) c -> p cb c", p=128)
                xb = pool.tile([128, 4, 512], BF16, tag="xb",
                               name=f"xb_{a}")
                nc.sync.dma_start(xb[:], xv)
                T[("xb", a)] = xb

            for i in range(N_IMG + 2):
                a = i
                b = i - 2

                if a < N_IMG:
                    dma_in(a)
                    ps1 = [pspool.tile([128, 512], F32, tag="ps",
                                       name=f"ps1_{a}_{r}") for r in range(4)]
                    T[("ps1", a)] = ps1
                if b >= 0:
                    ps3 = [pspool.tile([128, 512], F32, tag="ps",
                                       name=f"ps3_{b}_{r}") for r in range(4)]
                    T[("ps3", b)] = ps3

                # phase A: S3(b) + S1(a), interleaved per bank group
                for rb in range(4):
                    if b >= 0:
                        dt_ = T[("dt", b)]
                        ps3 = T[("ps3", b)]
                        for cb in range(4):
                            lhs = dt_[:, cb, 128 * rb:128 * rb + 128]
                            out = ps3[rb][:, 128 * cb:128 * cb + 128]
                            nc.tensor.matmul(out, lhs, Bhi[:],
                                             start=True, stop=False)
                            nc.tensor.matmul(out, lhs, Blo[:],
                                             start=False, stop=True)
                    if a < N_IMG:
                        xb = T[("xb", a)]
                        ps1 = T[("ps1", a)]
                        for cb in range(4):
                            nc.tensor.matmul(
                                ps1[rb][:, 128 * cb:128 * cb + 128],
                                xb[:, cb, 128 * rb:128 * rb + 128],
                                Abf[:], start=True, stop=True)

                # drain phase A psums
                if b >= 0:
                    wt = pool.tile([128, 4, 512], F32, tag="wt",
                                   name=f"wt_{b}")
                    T[("wt", b)] = wt
                    ps3 = T[("ps3", b)]
                    for rb in range(4):
                        nc.scalar.activation(
                            wt[:, rb, :], ps3[rb][:],
                            mybir.ActivationFunctionType.Identity,
                            bias=DCB[:])
                if a < N_IMG:
                    vt = pool.tile([128, 4, 512], BF16, tag="vt",
                                   name=f"vt_{a}")
                    T[("vt", a)] = vt
                    ps1 = T[("ps1", a)]
                    for rb in range(4):
                        nc.scalar.copy(vt[:, rb, :], ps1[rb][:])

                # phase B: S4(b) + S2(a) + rank-1 DC(a)
                if b >= 0:
                    ps4 = [pspool.tile([128, 512], F32, tag="ps",
                                       name=f"ps4_{b}_{r}") for r in range(4)]
                    T[("ps4", b)] = ps4
                    wt = T[("wt", b)]
                    for rb in range(4):
                        nc.tensor.matmul(ps4[rb][:], Bf[:], wt[:, rb, :],
                                         start=True, stop=True)
                if a < N_IMG:
                    ps2 = [pspool.tile([128, 512], F32, tag="ps",
                                       name=f"ps2_{a}_{r}") for r in range(4)]
                    T[("ps2", a)] = ps2
                    vt = T[("vt", a)]
                    for rb in range(4):
                        nc.tensor.matmul(ps2[rb][:], Abf[:], vt[:, rb, :],
                                         start=True, stop=False)
                    for rb in range(4):
                        nc.tensor.matmul(ps2[rb][:], DCU[:], DCV[:],
                                         start=False, stop=True)

                # drain phase B: clip+store (b), quant chain (a)
                if b >= 0:
                    rt = pool.tile([128, 4, 512], F32, tag="rt",
                                   name=f"rt_{b}")
                    ps4 = T[("ps4", b)]
                    for rb in range(4):
                        nc.scalar.activation(rt[:, rb, :], ps4[rb][:],
                                             mybir.ActivationFunctionType.Relu)
                    ot = pool.tile([128, 4, 512], F32, tag="ot",
                                   name=f"ot_{b}")
                    nc.vector.tensor_scalar_min(
                        ot[:].rearrange("p a b -> p (a b)"),
                        rt[:].rearrange("p a b -> p (a b)"), 1.0)
                    yv = y_d.ap()[b].rearrange("(cb p) c -> p cb c", p=128)
                    nc.sync.dma_start(yv, ot[:])
                if a < N_IMG:
                    ps2 = T[("ps2", a)]
                    nt = pool.tile([128, 4, 512], F32, tag="nt",
                                   name=f"nt_{a}")
                    for rb in range(4):
                        nc.vector.tensor_tensor(nt[:, rb, :], ps2[rb][:],
                                                QI[:, rb, :], op.mult)
                    tt = pool.tile([128, 4, 512], F32, tag="tt",
                                   name=f"tt_{a}")
                    for rb in range(4):
                        nc.vector.tensor_scalar(tt[:, rb, :], nt[:, rb, :],
                                                MAGIC, -MAGIC, op.add, op.add)
                    dt_ = pool.tile([128, 4, 512], BF16, tag="dt",
                                    name=f"dt_{a}")
                    T[("dt", a)] = dt_
                    for rb in range(4):
                        nc.gpsimd.tensor_tensor(dt_[:, rb, :], tt[:, rb, :],
                                                QQ[:, rb, :], op.mult)

    nc.compile()
    return nc, consts


_CACHE: dict = {}


def _get_program(quality: int):
    if quality not in _CACHE:
        _CACHE[quality] = build_program(quality)
    return _CACHE[quality]


def kernel(image: np.ndarray, quality) -> np.ndarray:
    from concourse.bass_utils import run_bass_kernel_spmd

    import ml_dtypes
    quality = int(quality)
    image = np.asarray(image, dtype=np.float32)
    image_bf = image.astype(ml_dtypes.bfloat16)
    nc, consts = _get_program(quality)

    shards = image_bf.reshape(N_CORES, N_IMG, H, W)
    base = {
        "abf": consts["Abf"],
        "b128": consts["B128"],
        "bhi": consts["Bhi"],
        "blo": consts["Blo"],
        "qinv": consts["QINV"].reshape(128, 4, 512),
        "qq": consts["QQ"].reshape(128, 4, 512),
        "dcb": consts["DCB"],
        "dcu": consts["DCU"],
        "dcv": consts["DCV"],
    }
    in_maps = [dict(base, x=np.ascontiguousarray(shards[c]))
               for c in range(N_CORES)]
    res = run_bass_kernel_spmd(nc, in_maps, core_ids=list(range(N_CORES)))
    out = np.stack([res.results[c]["y"] for c in range(N_CORES)])
    return out.reshape(B_FULL, C_CH, H, W)


# revision 31
# speedup vs baseline: 1.0766x; 1.0766x over previous
"""Differentiable JPEG on 8 Trainium2 NeuronCores (Bass/Tile).

Full inputs in, full outputs out. Data-parallel over the batch dim:
32 images -> 8 cores x 4 images (12 channel-planes of 512x512 each).

Per 512x512 plane P (values in [0,1]):
  Y^T = A^T (X^T A)          A = kron(I16, D^T)   (2D DCT, transposed layout;
                                                   both stages fp32 matmuls,
                                                   S1 uses the data as lhsT so
                                                   the transpose comes free)
  n   = Y^T * (1/q)          (DVE tensor_tensor, reads PSUM)
  n_dc -= 4/q00              (strided DC fix: folds the reference's x-0.5)
  r   = (n + C) - C          C = 1.5*2^23: fp32 magic-constant round-half-even
  d   = r * q                (GpSimd scalar_tensor_tensor, bf16 out -- exact:
                              |r|<=1 and q in {k/2} so r*q fits bf16)
  W   = d^T B                B = kron(I16, D), split B = Bhi+Blo in bf16
  W  += sqrt2 on rows p%8==0 (ACT Identity bias: folds the +4 DC / +0.5 shift)
  S   = B^T W                (fp32 matmul)
  out = min(relu(S), 1)
"""

import numpy as np

import concourse.bass as bass
import concourse.bacc as bacc
import concourse.mybir as mybir
import concourse.tile as tile

N_CORES = 8
B_FULL, C_CH, H, W = 32, 3, 512, 512
PER_CORE_B = B_FULL // N_CORES          # 4
N_IMG = PER_CORE_B * C_CH               # 12 planes of 512x512 per core
MAGIC = 12582912.0                      # 1.5 * 2**23
F32 = mybir.dt.float32
BF16 = mybir.dt.bfloat16

_JPEG_QUANT_TABLE = np.array([
    [16, 11, 10, 16, 24, 40, 51, 61],
    [12, 12, 14, 19, 26, 58, 60, 55],
    [14, 13, 16, 24, 40, 57, 69, 56],
    [14, 17, 22, 29, 51, 87, 80, 62],
    [18, 22, 37, 56, 68, 109, 103, 77],
    [24, 35, 55, 64, 81, 104, 113, 92],
    [49, 64, 78, 87, 103, 121, 120, 101],
    [72, 92, 95, 98, 112, 100, 103, 99]], dtype=np.float32)


def _dct_matrix():
    n = 8
    k = np.arange(n)[:, None].astype(np.float64)
    i = np.arange(n)[None, :].astype(np.float64)
    alpha = np.where(k == 0, np.sqrt(1.0 / n), np.sqrt(2.0 / n))
    return (alpha * np.cos(np.pi * (2 * i + 1) * k / (2 * n))).astype(np.float32)


def host_constants(quality: int):
    D = _dct_matrix()
    scale = 5000.0 / quality if quality < 50 else 200.0 - 2.0 * quality
    qt = np.clip((_JPEG_QUANT_TABLE * np.float32(scale)) / np.float32(100.0),
                 np.float32(1.0), np.float32(255.0)).astype(np.float32)
    qinv = (np.float32(1.0) / qt).astype(np.float32)

    import ml_dtypes
    I16 = np.eye(16, dtype=np.float32)
    A128 = np.kron(I16, D.T).astype(np.float32)          # S1 rhs / S2 lhsT
    B128 = np.kron(I16, D).astype(np.float32)            # S3 rhs / S4 lhsT
    Abf = A128.astype(ml_dtypes.bfloat16)
    Bhi = B128.astype(ml_dtypes.bfloat16)
    Blo = (B128 - Bhi.astype(np.float32)).astype(ml_dtypes.bfloat16)

    # [128, 4*512] patterns over the Y^T layout: [p, j] -> q[j%8, p%8]
    p_idx = np.arange(128) % 8
    j_idx = np.arange(4 * 512) % 8
    QINV = np.ascontiguousarray(qinv[np.ix_(j_idx, p_idx)].T)   # [128, 2048]
    QQ = np.ascontiguousarray(qt[np.ix_(j_idx, p_idx)].T)       # [128, 2048]

    # rank-1 DC shift injected into ps2: Y^T - 4*P4 (folds the x-0.5 input
    # shift). u (lhsT, [1,128]) and v (rhs, [1,512]) are bf16-exact.
    DCU = np.where(np.arange(128) % 8 == 0, -4.0, 0.0)[None, :] \
        .astype(ml_dtypes.bfloat16)
    DCV = np.where(np.arange(512) % 8 == 0, 1.0, 0.0)[None, :] \
        .astype(ml_dtypes.bfloat16)
    dcb_val = np.float32(0.5 / np.float64(D[0, 0]))              # = sqrt(2)
    DCB = np.where(np.arange(128) % 8 == 0, dcb_val,
                   np.float32(0)).astype(np.float32)[:, None]    # [128, 1]
    return dict(A128=A128, B128=B128, Abf=np.asarray(Abf),
                Bhi=np.asarray(Bhi), Blo=np.asarray(Blo),
                QINV=QINV, QQ=QQ, DCB=DCB,
                DCU=np.asarray(DCU), DCV=np.asarray(DCV))


def build_program(quality: int):
    """Build + compile the per-core Bass program. Returns (nc, const arrays)."""
    consts = host_constants(quality)
    nc = bacc.Bacc("TRN2", target_bir_lowering=False, debug=False,
                   num_devices=N_CORES)

    x_d = nc.dram_tensor("x", [N_IMG, H, W], BF16, kind="ExternalInput")
    y_d = nc.dram_tensor("y", [N_IMG, H, W], F32, kind="ExternalOutput")
    abf_d = nc.dram_tensor("abf", [128, 128], BF16, kind="ExternalInput")
    b_d = nc.dram_tensor("b128", [128, 128], F32, kind="ExternalInput")
    bhi_d = nc.dram_tensor("bhi", [128, 128], BF16, kind="ExternalInput")
    blo_d = nc.dram_tensor("blo", [128, 128], BF16, kind="ExternalInput")
    qi_d = nc.dram_tensor("qinv", [128, 4, 512], F32, kind="ExternalInput")
    qq_d = nc.dram_tensor("qq", [128, 4, 512], F32, kind="ExternalInput")
    db_d = nc.dram_tensor("dcb", [128, 1], F32, kind="ExternalInput")
    dcu_d = nc.dram_tensor("dcu", [1, 128], BF16, kind="ExternalInput")
    dcv_d = nc.dram_tensor("dcv", [1, 512], BF16, kind="ExternalInput")

    op = mybir.AluOpType

    with tile.TileContext(nc) as tc:
        with (
            tc.tile_pool(name="const", bufs=1) as cpool,
            tc.tile_pool(name="sb", bufs=3) as pool,
            tc.tile_pool(name="ps", bufs=8, space="PSUM") as pspool,
        ):
            Abf = cpool.tile([128, 128], BF16)
            Bf = cpool.tile([128, 128], F32)
            Bhi = cpool.tile([128, 128], BF16)
            Blo = cpool.tile([128, 128], BF16)
            QI = cpool.tile([128, 4, 512], F32)
            QQ = cpool.tile([128, 4, 512], F32)
            DCB = cpool.tile([128, 1], F32)
            DCU = cpool.tile([1, 128], BF16)
            DCV = cpool.tile([1, 512], BF16)
            nc.sync.dma_start(DCU[:], dcu_d.ap())
            nc.sync.dma_start(DCV[:], dcv_d.ap())
            nc.sync.dma_start(Abf[:], abf_d.ap())
            nc.sync.dma_start(Bf[:], b_d.ap())
            nc.sync.dma_start(Bhi[:], bhi_d.ap())
            nc.sync.dma_start(Blo[:], blo_d.ap())
            nc.sync.dma_start(QI[:], qi_d.ap())
            nc.sync.dma_start(QQ[:], qq_d.ap())
            nc.sync.dma_start(DCB[:], db_d.ap())

            # ---- software pipeline: tick i runs {S1,S2,quant} of image
            # a=i and {S3,S4,clip,out} of image b=i-2. Phase A uses
            # ps3(b)+ps1(a) = 8 banks, phase B ps4(b)+ps2(a) = 8 banks, so
            # the PE never waits on a PSUM slot and stays dense (HAM-warm).
            T = {}

            def dma_in(a):
                xv = x_d.ap()[a].rearrange("(cb p) c -> p cb c", p=128)
                xb = pool.tile([128, 4, 512], BF16, tag="xb",
                               name=f"xb_{a}")
                nc.sync.dma_start(xb[:], xv)
                T[("xb", a)] = xb

            for i in range(N_IMG + 2):
                a = i
                b = i - 2

                if a < N_IMG:
                    dma_in(a)
                    ps1 = [pspool.tile([128, 512], F32, tag="ps",
                                       name=f"ps1_{a}_{r}") for r in range(4)]
                    T[("ps1", a)] = ps1
                if b >= 0:
                    ps3 = [pspool.tile([128, 512], F32, tag="ps",
                                       name=f"ps3_{b}_{r}") for r in range(4)]
                    T[("ps3", b)] = ps3

                # phase A: S3(b) + S1(a), interleaved per bank group
                for rb in range(4):
                    if b >= 0:
                        dt_ = T[("dt", b)]
                        ps3 = T[("ps3", b)]
                        for cb in range(4):
                            lhs = dt_[:, cb, 128 * rb:128 * rb + 128]
                            out = ps3[rb][:, 128 * cb:128 * cb + 128]
                            nc.tensor.matmul(out, lhs, Bhi[:],
                                             start=True, stop=False)
                            nc.tensor.matmul(out, lhs, Blo[:],
                                             start=False, stop=True)
                    if a < N_IMG:
                        xb = T[("xb", a)]
                        ps1 = T[("ps1", a)]
                        for cb in range(4):
                            nc.tensor.matmul(
                                ps1[rb][:, 128 * cb:128 * cb + 128],
                                xb[:, cb, 128 * rb:128 * rb + 128],
                                Abf[:], start=True, stop=True)

                # drain phase A psums
                if b >= 0:
                    wt = pool.tile([128, 4, 512], F32, tag="wt",
                                   name=f"wt_{b}")
                    T[("wt", b)] = wt
                    ps3 = T[("ps3", b)]
                    for rb in range(4):
                        nc.scalar.activation(
                            wt[:, rb, :], ps3[rb][:],
                            mybir.ActivationFunctionType.Identity,
                            bias=DCB[:])
                if a < N_IMG:
                    vt = pool.tile([128, 4, 512], BF16, tag="vt",
                                   name=f"vt_{a}")
                    T[("vt", a)] = vt
                    ps1 = T[("ps1", a)]
                    for rb in range(4):
                        nc.scalar.copy(vt[:, rb, :], ps1[rb][:])

                # phase B: S4(b) + S2(a) + rank-1 DC(a)
                if b >= 0:
                    ps4 = [pspool.tile([128, 512], F32, tag="ps",
                                       name=f"ps4_{b}_{r}") for r in range(4)]
                    T[("ps4", b)] = ps4
                    wt = T[("wt", b)]
                    for rb in range(4):
                        nc.tensor.matmul(ps4[rb][:], Bf[:], wt[:, rb, :],
                                         start=True, stop=True)
                if a < N_IMG:
                    ps2 = [pspool.tile([128, 512], F32, tag="ps",
                                       name=f"ps2_{a}_{r}") for r in range(4)]
                    T[("ps2", a)] = ps2
                    vt = T[("vt", a)]
                    for rb in range(4):
                        nc.tensor.matmul(ps2[rb][:], Abf[:], vt[:, rb, :],
                                         start=True, stop=False)
                    for rb in range(4):
                        # rank-1 DC shift, strided: touches only n%8==0 cols
                        nc.tensor.matmul(
                            ps2[rb][:].rearrange("p (a e) -> p a e", e=8)[:, :, 0],
                            DCU[:], DCV[:].rearrange("p (a e) -> p a e", e=8)[:, :, 0],
                            start=False, stop=True)

                # drain phase B: clip+store (b), quant chain (a)
                if b >= 0:
                    ot = pool.tile([128, 4, 512], F32, tag="ot",
                                   name=f"ot_{b}")
                    ps4 = T[("ps4", b)]
                    for rb in range(4):
                        nc.vector.tensor_scalar(ot[:, rb, :], ps4[rb][:],
                                                0.0, 1.0, op.max, op.min)
                    yv = y_d.ap()[b].rearrange("(cb p) c -> p cb c", p=128)
                    nc.sync.dma_start(yv, ot[:])
                if a < N_IMG:
                    ps2 = T[("ps2", a)]
                    nt = pool.tile([128, 4, 512], F32, tag="nt",
                                   name=f"nt_{a}")
                    for rb in range(4):
                        nc.vector.tensor_tensor(nt[:, rb, :], ps2[rb][:],
                                                QI[:, rb, :], op.mult)
                    tt = pool.tile([128, 4, 512], F32, tag="tt",
                                   name=f"tt_{a}")
                    for rb in range(4):
                        nc.vector.tensor_scalar(tt[:, rb, :], nt[:, rb, :],
                                                MAGIC, -MAGIC, op.add, op.add)
                    dt_ = pool.tile([128, 4, 512], BF16, tag="dt",
                                    name=f"dt_{a}")
                    T[("dt", a)] = dt_
                    for rb in range(4):
                        nc.gpsimd.tensor_tensor(dt_[:, rb, :], tt[:, rb, :],
                                                QQ[:, rb, :], op.mult)

    nc.compile()
    return nc, consts


_CACHE: dict = {}


def _get_program(quality: int):
    if quality not in _CACHE:
        _CACHE[quality] = build_program(quality)
    return _CACHE[quality]


def kernel(image: np.ndarray, quality) -> np.ndarray:
    from concourse.bass_utils import run_bass_kernel_spmd

    import ml_dtypes
    quality = int(quality)
    image = np.asarray(image, dtype=np.float32)
    image_bf = image.astype(ml_dtypes.bfloat16)
    nc, consts = _get_program(quality)

    shards = image_bf.reshape(N_CORES, N_IMG, H, W)
    base = {
        "abf": consts["Abf"],
        "b128": consts["B128"],
        "bhi": consts["Bhi"],
        "blo": consts["Blo"],
        "qinv": consts["QINV"].reshape(128, 4, 512),
        "qq": consts["QQ"].reshape(128, 4, 512),
        "dcb": consts["DCB"],
        "dcu": consts["DCU"],
        "dcv": consts["DCV"],
    }
    in_maps = [dict(base, x=np.ascontiguousarray(shards[c]))
               for c in range(N_CORES)]
    res = run_bass_kernel_spmd(nc, in_maps, core_ids=list(range(N_CORES)))
    out = np.stack([res.results[c]["y"] for c in range(N_CORES)])
    return out.reshape(B_FULL, C_CH, H, W)


# revision 32
# speedup vs baseline: 1.1782x; 1.0944x over previous
"""Differentiable JPEG on 8 Trainium2 NeuronCores (Bass/Tile).

Full inputs in, full outputs out. Data-parallel over the batch dim:
32 images -> 8 cores x 4 images (12 channel-planes of 512x512 each).

Per 512x512 plane P (values in [0,1]):
  Y^T = A^T (X^T A)          A = kron(I16, D^T)   (2D DCT, transposed layout;
                                                   both stages fp32 matmuls,
                                                   S1 uses the data as lhsT so
                                                   the transpose comes free)
  n   = Y^T * (1/q)          (DVE tensor_tensor, reads PSUM)
  n_dc -= 4/q00              (strided DC fix: folds the reference's x-0.5)
  r   = (n + C) - C          C = 1.5*2^23: fp32 magic-constant round-half-even
  d   = r * q                (GpSimd scalar_tensor_tensor, bf16 out -- exact:
                              |r|<=1 and q in {k/2} so r*q fits bf16)
  W   = d^T B                B = kron(I16, D), split B = Bhi+Blo in bf16
  W  += sqrt2 on rows p%8==0 (ACT Identity bias: folds the +4 DC / +0.5 shift)
  S   = B^T W                (fp32 matmul)
  out = min(relu(S), 1)
"""

import numpy as np

import concourse.bass as bass
import concourse.bacc as bacc
import concourse.mybir as mybir
import concourse.tile as tile

N_CORES = 8
B_FULL, C_CH, H, W = 32, 3, 512, 512
PER_CORE_B = B_FULL // N_CORES          # 4
N_IMG = PER_CORE_B * C_CH               # 12 planes of 512x512 per core
MAGIC = 12582912.0                      # 1.5 * 2**23
F32 = mybir.dt.float32
BF16 = mybir.dt.bfloat16

_JPEG_QUANT_TABLE = np.array([
    [16, 11, 10, 16, 24, 40, 51, 61],
    [12, 12, 14, 19, 26, 58, 60, 55],
    [14, 13, 16, 24, 40, 57, 69, 56],
    [14, 17, 22, 29, 51, 87, 80, 62],
    [18, 22, 37, 56, 68, 109, 103, 77],
    [24, 35, 55, 64, 81, 104, 113, 92],
    [49, 64, 78, 87, 103, 121, 120, 101],
    [72, 92, 95, 98, 112, 100, 103, 99]], dtype=np.float32)


def _dct_matrix():
    n = 8
    k = np.arange(n)[:, None].astype(np.float64)
    i = np.arange(n)[None, :].astype(np.float64)
    alpha = np.where(k == 0, np.sqrt(1.0 / n), np.sqrt(2.0 / n))
    return (alpha * np.cos(np.pi * (2 * i + 1) * k / (2 * n))).astype(np.float32)


def host_constants(quality: int):
    D = _dct_matrix()
    scale = 5000.0 / quality if quality < 50 else 200.0 - 2.0 * quality
    qt = np.clip((_JPEG_QUANT_TABLE * np.float32(scale)) / np.float32(100.0),
                 np.float32(1.0), np.float32(255.0)).astype(np.float32)
    qinv = (np.float32(1.0) / qt).astype(np.float32)

    import ml_dtypes
    I16 = np.eye(16, dtype=np.float32)
    A128 = np.kron(I16, D.T).astype(np.float32)          # S1 rhs / S2 lhsT
    B128 = np.kron(I16, D).astype(np.float32)            # S3 rhs / S4 lhsT
    Abf = A128.astype(ml_dtypes.bfloat16)
    Bhi = B128.astype(ml_dtypes.bfloat16)
    Blo = (B128 - Bhi.astype(np.float32)).astype(ml_dtypes.bfloat16)

    # [128, 4*512] patterns over the Y^T layout: [p, j] -> q[j%8, p%8]
    p_idx = np.arange(128) % 8
    j_idx = np.arange(4 * 512) % 8
    QINV = np.ascontiguousarray(qinv[np.ix_(j_idx, p_idx)].T)   # [128, 2048]
    QQ = np.ascontiguousarray(qt[np.ix_(j_idx, p_idx)].T)       # [128, 2048]

    # rank-1 DC shift injected into ps2: Y^T - 4*P4 (folds the x-0.5 input
    # shift). u (lhsT, [1,128]) and v (rhs, [1,512]) are bf16-exact.
    DCU = np.where(np.arange(128) % 8 == 0, -4.0, 0.0)[None, :] \
        .astype(ml_dtypes.bfloat16)
    DCV = np.where(np.arange(512) % 8 == 0, 1.0, 0.0)[None, :] \
        .astype(ml_dtypes.bfloat16)
    dcb_val = np.float32(0.5 / np.float64(D[0, 0]))              # = sqrt(2)
    DCB = np.where(np.arange(128) % 8 == 0, dcb_val,
                   np.float32(0)).astype(np.float32)[:, None]    # [128, 1]
    return dict(A128=A128, B128=B128, Abf=np.asarray(Abf),
                Bhi=np.asarray(Bhi), Blo=np.asarray(Blo),
                QINV=QINV, QQ=QQ, DCB=DCB,
                DCU=np.asarray(DCU), DCV=np.asarray(DCV))


def build_program(quality: int):
    """Build + compile the per-core Bass program. Returns (nc, const arrays)."""
    consts = host_constants(quality)
    nc = bacc.Bacc("TRN2", target_bir_lowering=False, debug=False,
                   num_devices=N_CORES)

    x_d = nc.dram_tensor("x", [N_IMG, H, W], BF16, kind="ExternalInput")
    y_d = nc.dram_tensor("y", [N_IMG, H, W], F32, kind="ExternalOutput")
    abf_d = nc.dram_tensor("abf", [128, 128], BF16, kind="ExternalInput")
    b_d = nc.dram_tensor("b128", [128, 128], F32, kind="ExternalInput")
    bhi_d = nc.dram_tensor("bhi", [128, 128], BF16, kind="ExternalInput")
    blo_d = nc.dram_tensor("blo", [128, 128], BF16, kind="ExternalInput")
    qi_d = nc.dram_tensor("qinv", [128, 4, 512], F32, kind="ExternalInput")
    qq_d = nc.dram_tensor("qq", [128, 4, 512], F32, kind="ExternalInput")
    db_d = nc.dram_tensor("dcb", [128, 1], F32, kind="ExternalInput")
    dcu_d = nc.dram_tensor("dcu", [1, 128], BF16, kind="ExternalInput")
    dcv_d = nc.dram_tensor("dcv", [1, 512], BF16, kind="ExternalInput")

    op = mybir.AluOpType

    with tile.TileContext(nc) as tc:
        with (
            tc.tile_pool(name="const", bufs=1) as cpool,
            tc.tile_pool(name="sb", bufs=3) as pool,
            tc.tile_pool(name="ps", bufs=8, space="PSUM") as pspool,
        ):
            Abf = cpool.tile([128, 128], BF16)
            Bf = cpool.tile([128, 128], F32)
            Bhi = cpool.tile([128, 128], BF16)
            Blo = cpool.tile([128, 128], BF16)
            QI = cpool.tile([128, 4, 512], F32)
            QQ = cpool.tile([128, 4, 512], F32)
            DCB = cpool.tile([128, 1], F32)
            DCU = cpool.tile([1, 128], BF16)
            DCV = cpool.tile([1, 512], BF16)

            # ---- software pipeline: tick i runs {S1,S2,quant} of image
            # a=i and {S3,S4,clip,out} of image b=i-2. Phase A uses
            # ps3(b)+ps1(a) = 8 banks, phase B ps4(b)+ps2(a) = 8 banks, so
            # the PE never waits on a PSUM slot and stays dense (HAM-warm).
            T = {}
            _consts_emitted = [False]

            def emit_consts():
                # after xb(0): order by first use so tick-0 never stalls
                nc.sync.dma_start(Abf[:], abf_d.ap())
                nc.sync.dma_start(DCU[:], dcu_d.ap())
                nc.sync.dma_start(DCV[:], dcv_d.ap())
                nc.sync.dma_start(QI[:], qi_d.ap())
                nc.sync.dma_start(QQ[:], qq_d.ap())
                nc.sync.dma_start(Bhi[:], bhi_d.ap())
                nc.sync.dma_start(Blo[:], blo_d.ap())
                nc.sync.dma_start(DCB[:], db_d.ap())
                nc.sync.dma_start(Bf[:], b_d.ap())

            def dma_in(a):
                xv = x_d.ap()[a].rearrange("(cb p) c -> p cb c", p=128)
                xb = pool.tile([128, 4, 512], BF16, tag="xb",
                               name=f"xb_{a}")
                nc.sync.dma_start(xb[:], xv)
                T[("xb", a)] = xb

            for i in range(N_IMG + 2):
                a = i
                b = i - 2

                if a < N_IMG:
                    dma_in(a)
                    if not _consts_emitted[0]:
                        emit_consts()
                        _consts_emitted[0] = True
                    ps1 = [pspool.tile([128, 512], F32, tag="ps",
                                       name=f"ps1_{a}_{r}") for r in range(4)]
                    T[("ps1", a)] = ps1
                if b >= 0:
                    ps3 = [pspool.tile([128, 512], F32, tag="ps",
                                       name=f"ps3_{b}_{r}") for r in range(4)]
                    T[("ps3", b)] = ps3

                # phase A: S3(b) + S1(a), interleaved per bank group
                for rb in range(4):
                    if b >= 0:
                        dt_ = T[("dt", b)]
                        ps3 = T[("ps3", b)]
                        for cb in range(4):
                            lhs = dt_[:, cb, 128 * rb:128 * rb + 128]
                            out = ps3[rb][:, 128 * cb:128 * cb + 128]
                            nc.tensor.matmul(out, lhs, Bhi[:],
                                             start=True, stop=False)
                            nc.tensor.matmul(out, lhs, Blo[:],
                                             start=False, stop=True)
                    if a < N_IMG:
                        xb = T[("xb", a)]
                        ps1 = T[("ps1", a)]
                        for cb in range(4):
                            nc.tensor.matmul(
                                ps1[rb][:, 128 * cb:128 * cb + 128],
                                xb[:, cb, 128 * rb:128 * rb + 128],
                                Abf[:], start=True, stop=True)

                # drain phase A psums
                if b >= 0:
                    wt = pool.tile([128, 4, 512], F32, tag="wt",
                                   name=f"wt_{b}")
                    T[("wt", b)] = wt
                    ps3 = T[("ps3", b)]
                    for rb in range(4):
                        nc.scalar.activation(
                            wt[:, rb, :], ps3[rb][:],
                            mybir.ActivationFunctionType.Identity,
                            bias=DCB[:])
                if a < N_IMG:
                    vt = pool.tile([128, 4, 512], BF16, tag="vt",
                                   name=f"vt_{a}")
                    T[("vt", a)] = vt
                    ps1 = T[("ps1", a)]
                    for rb in range(4):
                        nc.scalar.copy(vt[:, rb, :], ps1[rb][:])

                # phase B: S4(b) + S2(a) + rank-1 DC(a)
                if b >= 0:
                    ps4 = [pspool.tile([128, 512], F32, tag="ps",
                                       name=f"ps4_{b}_{r}") for r in range(4)]
                    T[("ps4", b)] = ps4
                    wt = T[("wt", b)]
                    for rb in range(4):
                        nc.tensor.matmul(ps4[rb][:], Bf[:], wt[:, rb, :],
                                         start=True, stop=True)
                if a < N_IMG:
                    ps2 = [pspool.tile([128, 512], F32, tag="ps",
                                       name=f"ps2_{a}_{r}") for r in range(4)]
                    T[("ps2", a)] = ps2
                    vt = T[("vt", a)]
                    for rb in range(4):
                        nc.tensor.matmul(ps2[rb][:], Abf[:], vt[:, rb, :],
                                         start=True, stop=False)
                    for rb in range(4):
                        # rank-1 DC shift, strided: touches only n%8==0 cols
                        nc.tensor.matmul(
                            ps2[rb][:].rearrange("p (a e) -> p a e", e=8)[:, :, 0],
                            DCU[:], DCV[:].rearrange("p (a e) -> p a e", e=8)[:, :, 0],
                            start=False, stop=True)

                # drain phase B: clip+store (b), quant chain (a)
                if b >= 0:
                    ot = pool.tile([128, 4, 512], F32, tag="ot",
                                   name=f"ot_{b}")
                    ps4 = T[("ps4", b)]
                    yv = y_d.ap()[b].rearrange("(cb p) c -> p cb c", p=128)
                    for rb in range(4):
                        nc.vector.tensor_scalar(ot[:, rb, :], ps4[rb][:],
                                                0.0, 1.0, op.max, op.min)
                        nc.sync.dma_start(yv[:, rb, :], ot[:, rb, :])
                if a < N_IMG:
                    ps2 = T[("ps2", a)]
                    nt = pool.tile([128, 4, 512], F32, tag="nt",
                                   name=f"nt_{a}")
                    for rb in range(4):
                        nc.vector.tensor_tensor(nt[:, rb, :], ps2[rb][:],
                                                QI[:, rb, :], op.mult)
                    tt = pool.tile([128, 4, 512], F32, tag="tt",
                                   name=f"tt_{a}")
                    for rb in range(4):
                        nc.vector.tensor_scalar(tt[:, rb, :], nt[:, rb, :],
                                                MAGIC, -MAGIC, op.add, op.add)
                    dt_ = pool.tile([128, 4, 512], BF16, tag="dt",
                                    name=f"dt_{a}")
                    T[("dt", a)] = dt_
                    nc.vector.scalar_tensor_tensor(
                        dt_[:].rearrange("p a b -> p (a b)"),
                        tt[:].rearrange("p a b -> p (a b)"), 0.0,
                        QQ[:].rearrange("p a b -> p (a b)"), op.add, op.mult)

    nc.compile()
    return nc, consts


_CACHE: dict = {}


def _get_program(quality: int):
    if quality not in _CACHE:
        _CACHE[quality] = build_program(quality)
    return _CACHE[quality]


def kernel(image: np.ndarray, quality) -> np.ndarray:
    from concourse.bass_utils import run_bass_kernel_spmd

    import ml_dtypes
    quality = int(quality)
    image = np.asarray(image, dtype=np.float32)
    image_bf = image.astype(ml_dtypes.bfloat16)
    nc, consts = _get_program(quality)

    shards = image_bf.reshape(N_CORES, N_IMG, H, W)
    base = {
        "abf": consts["Abf"],
        "b128": consts["B128"],
        "bhi": consts["Bhi"],
        "blo": consts["Blo"],
        "qinv": consts["QINV"].reshape(128, 4, 512),
        "qq": consts["QQ"].reshape(128, 4, 512),
        "dcb": consts["DCB"],
        "dcu": consts["DCU"],
        "dcv": consts["DCV"],
    }
    in_maps = [dict(base, x=np.ascontiguousarray(shards[c]))
               for c in range(N_CORES)]
    res = run_bass_kernel_spmd(nc, in_maps, core_ids=list(range(N_CORES)))
    out = np.stack([res.results[c]["y"] for c in range(N_CORES)])
    return out.reshape(B_FULL, C_CH, H, W)
